# revision 30
# baseline (speedup 1.0000x reference)
"""Longformer-style blocked local+global attention on 8 Trainium2 NeuronCores.

Problem (hardcoded): B=2, S=4096, D=1024, H=16, DH=64, W=256 (block size =
one-sided window radius), G=64 global prefix tokens.

Sharding: batch x head-group. Core c handles batch b = c//4 and heads
[(c%4)*4, (c%4)*4+4). Everything for one (b, head-group) is independent, so
there are no collectives; the only cross-core interaction is the final
output-projection partial sum, which the host performs (4 partials per batch).

Per-core kernel layout strategy:
  - Host passes x[b] pre-transposed (xT = [D, S]) so all projection matmuls
    have their contraction dim (D) on SBUF partitions.
  - q, k are produced directly in transposed layout qT/kT = [head_dim, S].
  - v is produced in natural layout [S, DH] per head, with a 65th column of
    ones appended so the PV matmul accumulates the softmax denominator as
    row 64 of the output for free.
  - Scores are computed transposed (sT = [key_pos, query]) so that exp'd
    scores feed the PV matmul directly as the moving operand -- no
    transposes anywhere in the kernel.
  - Softmax uses exp without max subtraction (scores are O(1) by
    construction), and masked entries are zeroed via precomputed 0/1
    triangular mask tiles after exp.
  - All matmul inputs are bf16; accumulation is fp32 in PSUM.

Scheduling (the PE HAM clock gate demotes the PE to 1.2 GHz after ~3.4us of
idle and only re-promotes after ~3.4us of sustained activity, so the tensor
stream must never develop bubbles):
  - Startup: wq is loaded per-k-tile so the first projection matmuls start
    early; wk/wv/wo/masks are single DMAs behind it on the gpsimd queue; each
    xT chunk is split across the sync and scalar queues.
  - Phase 2 per block n emits scores(n,hp0), scores(n,hp1), Wo(n-1),
    PV(n,hp0)+norm, PV(n,hp1)+norm.  Wo and the second PV cover the exp,
    mask and normalize latencies of the first.
  - Score tiles go out c3/c4 first (the first chunks PV consumes); PV runs
    its mask-free/early chunks first and the [c0|c5|glb] tail last.
  - et is [128, ti, hh, 512] so the three per-(hp,hh) triangle masks of a
    generic block collapse into ONE strided 768-col multiply; hh0 runs on
    DVE and hh1 on Pool(gpsimd) to balance the queues.
  - Per-block engine budget vs the PE's 6.8us: ACT 12 exps (~6.9);
    DVE combined-mask hh0 + den copies + reciprocals + normalize muls +
    Wo psum->sbuf bounces (~6.7); Pool combined-mask hh1 + den broadcasts
    (~4); every chain gets >=2.5us of PE cover before it is consumed.
  - Score psum tiles and Wo psum tiles share one 6-buffer pool tag; the two
    PV accumulators double-buffer: 6 + 2 = all 8 psum banks.
"""

import numpy as np
import ml_dtypes

import concourse.bacc as bacc
import concourse.bass as bass
import concourse.mybir as mybir
import concourse.tile as tile
from concourse.bass_utils import run_bass_kernel_spmd

BF16 = mybir.dt.bfloat16
F32 = mybir.dt.float32
NPBF = ml_dtypes.bfloat16

B, S, D = 2, 4096, 1024
H, DH = 16, 64
W = 256          # block size == window radius
G = 64           # global prefix tokens
NB = S // W      # 16 blocks
SCALE = 1.0 / 8.0  # 1/sqrt(DH)

N_CORES = 8
HEADS_PER_CORE = 4
ECOLS = HEADS_PER_CORE * DH   # 256 embedding columns per core

# mask stack indices (each [128, 512] left-aligned, see build_masks)
M_L1, M_R0, M_EGEN, M_EN1, M_GC, M_R0R1, M_L0L1 = range(7)

# module-level caches
_BUILT = {}
LAST_RESULTS = None


def build_masks():
    """Returns (masks [7,128,512], cmasks [2,128,768]) bf16 0/1 tiles.

    Local-strip chunk c of query block n holds key rows kj of block n-1
    (c=0,1), n (c=2,3), n+1 (c=4,5). Triangle masks (r = row within chunk,
    q = query within block): c0: q<=r (only q<128 possible); c1: q<=128+r;
    c4: q>=r; c5: q>=128+r (only q>=128 possible).

    cmasks are the generic-block combined masks, one 256-col slab per score
    tile in emission order [t(c4|c3) | t(c1|c2) | t(c0|c5|glb)]:
    [R0 | L1 | L0h,R1h], with the n==1 variant applying the global cut to
    the c0 triangle.
    """
    r = np.arange(128)[:, None]
    q = np.arange(256)[None, :]
    L0 = (q <= r).astype(np.float32)          # use cols 0:128
    L1 = (q <= 128 + r).astype(np.float32)
    R0 = (q >= r).astype(np.float32)
    R1 = (q >= 128 + r).astype(np.float32)    # use cols 128:256
    L0g = L0 * (r >= G)                       # left-upper chunk w/ global cut
    Gc = np.broadcast_to((r >= G).astype(np.float32), (128, 256)).copy()

    L0h, L0gh, R1h = L0[:, 0:128], L0g[:, 0:128], R1[:, 128:256]

    def pad(*parts):
        m = np.concatenate(parts, axis=1)
        if m.shape[1] < 512:
            m = np.concatenate(
                [m, np.zeros((128, 512 - m.shape[1]), np.float32)], axis=1)
        return m

    m = np.stack([
        pad(L1),             # M_L1
        pad(R0),             # M_R0
        pad(L0h, R1h),       # M_EGEN  [c0|c5]
        pad(L0gh, R1h),      # M_EN1   [c0 w/ global cut|c5] (n==1)
        pad(Gc),             # M_GC    (n==0 c2)
        pad(R0, R1h),        # M_R0R1  (n==0 [c4|c5])
        pad(L0h, L1),        # M_L0L1  (n==15 [c0|c1])
    ]).astype(NPBF)

    egen = np.concatenate([L0h, R1h], axis=1)   # [128, 256]
    en1 = np.concatenate([L0gh, R1h], axis=1)
    cm = np.stack([
        np.concatenate([R0, L1, egen], axis=1),  # generic
        np.concatenate([R0, L1, en1], axis=1),   # n == 1
    ]).astype(NPBF)
    return m, cm


def _tiles_for_block(n):
    """Per-head score-tile packing for query block n.

    Each head's active strip chunks (+ the global-prefix scores) are packed
    into three [128, 512] PSUM banks. Returns (tiles, cmask_idx):
      parts: [(c, col0, width, q0)] -- c in 0..5 or 'glb'; the chunk's
             scores land at psum/et cols [col0, col0+width), covering query
             range [q0, q0+width)
      exps:  [(col0, col1, rows)] -- merged exp ranges
      mask:  (col0, col1, mask_idx) or None -- per-tile mask multiply (edge
             blocks only; generic blocks use the combined cmask instead)
    cmask_idx is None for edge blocks, else the cmasks row to apply over
    et[:, :, hh, 0:256].  Chunk c covers sequence tile s_tile = 2*(n-1)+c.
    Generic blocks emit the c4/c3 tile first (c3 is the first chunk PV
    consumes) with the masked chunk at cols 0:256 so the three masks line
    up for one strided multiply.
    """
    if n == 0:
        return [
            dict(parts=[(2, 0, 256, 0), (3, 256, 256, 0)],
                 exps=[(0, 512, 128)], mask=(0, 256, M_GC)),
            dict(parts=[(4, 0, 256, 0), (5, 256, 128, 128)],
                 exps=[(0, 384, 128)], mask=(0, 384, M_R0R1)),
            dict(parts=[("glb", 0, 256, 0)], exps=[(0, 256, 64)], mask=None),
        ], None
    if n == NB - 1:
        return [
            dict(parts=[(0, 0, 128, 0), (1, 128, 256, 0)],
                 exps=[(0, 384, 128)], mask=(0, 384, M_L0L1)),
            dict(parts=[(2, 0, 256, 0), (3, 256, 256, 0)],
                 exps=[(0, 512, 128)], mask=None),
            dict(parts=[("glb", 0, 256, 0)], exps=[(0, 256, 64)], mask=None),
        ], None
    return [
        dict(parts=[(4, 0, 256, 0), (3, 256, 256, 0)],
             exps=[(0, 512, 128)], mask=None),
        dict(parts=[(1, 0, 256, 0), (2, 256, 256, 0)],
             exps=[(0, 512, 128)], mask=None),
        dict(parts=[(0, 0, 128, 0), (5, 128, 128, 128), ("glb", 256, 256, 0)],
             exps=[(0, 512, 128)], mask=None),
    ], (1 if n == 1 else 0)


def build():
    """Build the per-core Bass/Tile program (identical on all 8 cores)."""
    nc = bacc.Bacc("TRN2", target_bir_lowering=False, debug=False)

    xT = nc.dram_tensor("xT", [D, S], BF16, kind="ExternalInput")
    wq = nc.dram_tensor("wq", [D, ECOLS], BF16, kind="ExternalInput")
    wk = nc.dram_tensor("wk", [D, ECOLS], BF16, kind="ExternalInput")
    wv = nc.dram_tensor("wv", [D, ECOLS], BF16, kind="ExternalInput")
    wo = nc.dram_tensor("wo", [ECOLS, D], BF16, kind="ExternalInput")
    masks = nc.dram_tensor("masks", [7, 128, 512], BF16, kind="ExternalInput")
    cmasks = nc.dram_tensor("cmasks", [2, 128, 768], BF16,
                            kind="ExternalInput")
    y = nc.dram_tensor("y", [S, D], F32, kind="ExternalOutput")

    EXP = mybir.ActivationFunctionType.Exp

    with tile.TileContext(nc) as tc:
        with (
            tc.tile_pool(name="const", bufs=1) as constp,
            tc.tile_pool(name="persist", bufs=1) as pers,
            tc.tile_pool(name="etp", bufs=4) as etp,
            tc.tile_pool(name="attnp", bufs=4) as atp,
            tc.tile_pool(name="smallp", bufs=8) as smp,
            tc.tile_pool(name="yp", bufs=2) as yp,
        ):
            # ---- constants ----
            wq_sb = constp.tile([128, 8, ECOLS], BF16, name="wq_sb")
            wk_sb = constp.tile([128, 8, ECOLS], BF16, name="wk_sb")
            wv_sb = constp.tile([128, 8, ECOLS], BF16, name="wv_sb")
            wo_sb = constp.tile([128, 2, D], BF16, name="wo_sb")
            mk_sb = constp.tile([128, 7, 512], BF16, name="mk_sb")
            cm_sb = constp.tile([128, 2, 3, 256], BF16, name="cm_sb")
            for h in range(2):  # two halves so the first matmuls start early
                nc.gpsimd.dma_start(
                    out=wq_sb[:, h * 4:(h + 1) * 4, :],
                    in_=wq.ap()[h * 512:(h + 1) * 512, :]
                        .rearrange("(k p) e -> p k e", p=128))
            nc.gpsimd.dma_start(
                out=wk_sb[:], in_=wk.ap().rearrange("(k p) e -> p k e", p=128))
            nc.gpsimd.dma_start(
                out=wv_sb[:], in_=wv.ap().rearrange("(k p) e -> p k e", p=128))
            nc.gpsimd.dma_start(
                out=wo_sb[:], in_=wo.ap().rearrange("(e p) d -> p e d", p=128))
            nc.gpsimd.dma_start(
                out=mk_sb[:], in_=masks.ap().rearrange("m p q -> p m q"))
            nc.gpsimd.dma_start(
                out=cm_sb[:],
                in_=cmasks.ap().rearrange("g p (t q) -> p g t q", t=3))

            # ---- persistent per-head tensors ----
            qT = [pers.tile([128, S], BF16, name=f"qT{i}") for i in range(2)]
            kT = [pers.tile([128, S], BF16, name=f"kT{i}") for i in range(2)]
            # v natural layout: [128 seq-part, 32 seq-tiles, 4 heads, 65]
            # (col 64 = ones for the denominator row)
            vv = pers.tile([128, S // 128, HEADS_PER_CORE, 65], BF16, name="vv")
            for h in range(HEADS_PER_CORE):
                nc.vector.memset(vv[:, :, h, 64:65], 1.0)

            # ---- interleaved projections + attention ----
            # Projection chunks and attention blocks share one PE stream so
            # the exp/mask/normalize engines (busy only during attention)
            # spread over the whole kernel instead of cramming into a
            # second phase: block n is emitted as soon as its q/k/v columns
            # (chunks <= (n+1)//2) exist.  All projection psum tiles join
            # the score/Wo rotation (tag "a"): 6 + 2 PV banks = 8.
            with (
                tc.tile_pool(name="xstream", bufs=3) as xp,
                tc.tile_pool(name="ps_s", bufs=6, space="PSUM") as ps_sp,
                tc.tile_pool(name="ps_o", bufs=2, space="PSUM") as ps_op,
            ):
                def emit_chunk_qk(c):
                    xt = xp.tile([128, 8, 512], BF16, name="xt")
                    # split the chunk load across two queues so the first
                    # k-tiles land (and the first matmuls start) early
                    nc.sync.dma_start(
                        out=xt[:, 0:4, :],
                        in_=xT.ap()[0:512, c * 512:(c + 1) * 512]
                            .rearrange("(k p) s -> p k s", p=128))
                    nc.scalar.dma_start(
                        out=xt[:, 4:8, :],
                        in_=xT.ap()[512:1024, c * 512:(c + 1) * 512]
                            .rearrange("(k p) s -> p k s", p=128))
                    for hp in range(2):
                        for wsb, dst in ((wq_sb, qT[hp]), (wk_sb, kT[hp])):
                            ps = ps_sp.tile([128, 512], F32, name="ps_qk",
                                            tag="a")
                            for k in range(8):
                                nc.tensor.matmul(
                                    ps[:],
                                    wsb[:, k, hp * 128:(hp + 1) * 128],
                                    xt[:, k, :],
                                    start=(k == 0), stop=(k == 7))
                            nc.vector.tensor_copy(
                                dst[:, c * 512:(c + 1) * 512], ps[:])
                    return xt

                def emit_chunk_v(c, xt):
                    for ss in range(4):  # 128-row seq subtiles -> v natural
                        ps = ps_sp.tile([128, 512], F32, name="ps_v",
                                        tag="a")
                        for k in range(8):
                            nc.tensor.matmul(
                                ps[:, 0:ECOLS],
                                xt[:, k, ss * 128:(ss + 1) * 128],
                                wv_sb[:, k, :],
                                start=(k == 0), stop=(k == 7))
                        nc.vector.tensor_copy(
                            vv[:, c * 4 + ss, :, 0:64],
                            ps[:, 0:ECOLS].rearrange("p (h e) -> p h e", h=4))

                def emit_chunk(c):
                    emit_chunk_v(c, emit_chunk_qk(c))

                def emit_scores(n, hp, tiles, cidx, et, loc):
                    qpair, kpair = qT[hp], kT[hp]
                    for ti, sp in enumerate(tiles):
                        for hh in range(2):
                            hr = hh * 64
                            st = ps_sp.tile([128, 512], F32, name="st",
                                            tag="a")
                            for c, col0, width, q0 in sp["parts"]:
                                if c == "glb":
                                    lhs = kpair[hr:hr + 64, 0:G]
                                    rows = 64
                                else:
                                    s0 = (2 * (n - 1) + c) * 128
                                    lhs = kpair[hr:hr + 64, s0:s0 + 128]
                                    rows = 128
                                nc.tensor.matmul(
                                    st[0:rows, col0:col0 + width],
                                    lhs,
                                    qpair[hr:hr + 64,
                                          n * 256 + q0:
                                          n * 256 + q0 + width],
                                    start=True, stop=True)
                                if hh == 0:
                                    loc[c] = (ti, col0, width, q0)
                            for c0e, c1e, rows in sp["exps"]:
                                nc.scalar.activation(
                                    et[0:rows, ti, hh, c0e:c1e],
                                    st[0:rows, c0e:c1e], EXP, scale=SCALE)
                        if sp["mask"] is not None:  # edge blocks
                            m0, m1, mi = sp["mask"]
                            for hh in range(2):
                                nc.vector.tensor_mul(
                                    et[:, ti, hh, m0:m1],
                                    et[:, ti, hh, m0:m1],
                                    mk_sb[:, mi, 0:m1 - m0])
                    if cidx is not None:
                        # one strided 768-col multiply covers all three
                        # tiles' triangle masks
                        for hh in range(2):
                            nc.vector.tensor_mul(
                                et[:, :, hh, 0:256],
                                et[:, :, hh, 0:256],
                                cm_sb[:, cidx, :, :])

                def emit_pv(n, hp, et, loc, at_blk):
                    # PV + normalize; the two heads share one psum bank.
                    # Mask-free/early chunks first, the t2-dependent tail
                    # (c0/c5/glb) last so the combined mask has PE cover.
                    # psum groups must stay sequential per head: a group
                    # start clears has_written beyond its own address range,
                    # so interleaving the two heads' groups corrupts sums
                    pv_order = [c for c in (3, 2, 1, 4, 0, 5) if c in loc]
                    ot = ps_op.tile([128, 512], F32, name="ot", tag="ot")
                    for hh in range(2):
                        h = hp * 2 + hh
                        ob = hh * 256
                        for i, c in enumerate(pv_order):
                            ti, col0, width, q0 = loc[c]
                            s_tile = 2 * (n - 1) + c
                            nc.tensor.matmul(
                                ot[0:65, ob + q0:ob + q0 + width],
                                vv[:, s_tile, h, :],
                                et[:, ti, hh, col0:col0 + width],
                                start=(i == 0), stop=False)
                        ti, col0, width, q0 = loc["glb"]
                        nc.tensor.matmul(
                            ot[0:65, ob:ob + 256],
                            vv[0:64, 0, h, :],
                            et[0:64, ti, hh, col0:col0 + width],
                            start=False, stop=True)
                    # normalize: reciprocal_approx_fast needs exact fp32
                    # bits; its PSUM read path perturbs them (HW-measured
                    # ~5% error), so bounce the den row through SBUF.
                    den = smp.tile([1, 512], F32, name="den")
                    nc.vector.tensor_copy(den[:], ot[64:65, 0:512])
                    rec = smp.tile([1, 512], F32, name="rec")
                    nc.vector.reciprocal_approx_fast(rec[:], den[:])
                    recb = smp.tile([64, 512], F32, name="recb")
                    nc.gpsimd.partition_broadcast(recb[:], rec[:])
                    for hh in range(2):
                        ob = hh * 256
                        nc.vector.tensor_mul(
                            at_blk[hh * 64:(hh + 1) * 64, hp, :],
                            ot[0:64, ob:ob + 256], recb[:, ob:ob + 256])

                def emit_wo(n, at_blk):
                    """Output projection for block n's 256 rows."""
                    for ss in range(2):
                        ysb = yp.tile([128, D], F32, name="ysb")
                        for dk in range(2):
                            py_ = ps_sp.tile([128, 512], F32, name="py",
                                             tag="a")
                            for e in range(2):
                                nc.tensor.matmul(
                                    py_[:],
                                    at_blk[:, e, ss * 128:(ss + 1) * 128],
                                    wo_sb[:, e, dk * 512:(dk + 1) * 512],
                                    start=(e == 0), stop=(e == 1))
                            # scalar engine: pure f32 copy, keeps the DVE
                            # FIFO short for the mask/normalize chains
                            nc.scalar.copy(
                                ysb[:, dk * 512:(dk + 1) * 512], py_[:])
                        r0 = n * 256 + ss * 128
                        nc.sync.dma_start(out=y.ap()[r0:r0 + 128, :],
                                          in_=ysb[:])

                # Wo for block n runs between block n+1's two PVs so the
                # in-order PE never stalls on the normalize chain and the
                # second PV's exps/masks get extra cover.
                state = {"pending": None}

                def emit_block(n, mid=None):
                    at_blk = atp.tile([128, 2, 256], BF16, name="at_blk")
                    tiles, cidx = _tiles_for_block(n)
                    loc = {}
                    ets = []
                    for hp in range(2):
                        et = etp.tile([128, 3, 2, 512], BF16, name="et")
                        ets.append(et)
                        emit_scores(n, hp, tiles, cidx, et, loc)
                        if hp == 0 and mid is not None:
                            mid()  # chunk V matmuls drain hp0's exps
                    emit_pv(n, 0, ets[0], loc, at_blk)
                    if state["pending"] is not None:
                        emit_wo(*state["pending"])
                    emit_pv(n, 1, ets[1], loc, at_blk)
                    state["pending"] = (n, at_blk)

                # block n's inputs exist once chunk (n+1)//2 is in.  Blocks
                # sandwich each chunk ([B, c, B]) so the activation engine's
                # exp bursts spread over the chunk's projection matmuls
                # instead of cramming between consecutive blocks.
                emit_chunk(0)
                emit_block(0)
                emit_chunk(1)
                emit_block(1)
                for c in range(2, 8):
                    emit_block(2 * c - 2)   # needs chunk c-1 only
                    xt = emit_chunk_qk(c)
                    # block 2c-1 needs chunk c's q/k for scores but its v
                    # only at PV time: the V matmuls sit between the two
                    # score groups as an exp-drain window
                    emit_block(2 * c - 1,
                               mid=lambda c=c, xt=xt: emit_chunk_v(c, xt))
                emit_block(14)
                emit_block(15)
                emit_wo(*state["pending"])

    nc.compile()
    return nc


def _get_nc():
    if "nc" not in _BUILT:
        _BUILT["nc"] = build()
    return _BUILT["nc"]


def make_in_maps(x, Wq, Wk, Wv, Wo):
    masks_np, cmasks_np = build_masks()
    xT = [np.ascontiguousarray(x[b].T).astype(NPBF) for b in range(B)]
    wq16, wk16, wv16 = (w.astype(NPBF) for w in (Wq, Wk, Wv))
    wo16 = Wo.astype(NPBF)
    in_maps = []
    for core in range(N_CORES):
        b, hg = core // 4, core % 4
        cols = slice(hg * ECOLS, (hg + 1) * ECOLS)
        in_maps.append({
            "xT": xT[b],
            "wq": np.ascontiguousarray(wq16[:, cols]),
            "wk": np.ascontiguousarray(wk16[:, cols]),
            "wv": np.ascontiguousarray(wv16[:, cols]),
            "wo": np.ascontiguousarray(wo16[cols, :]),
            "masks": masks_np,
            "cmasks": cmasks_np,
        })
    return in_maps


def kernel(x, Wq, Wk, Wv, Wo):
    global LAST_RESULTS
    nc = _get_nc()
    in_maps = make_in_maps(x, Wq, Wk, Wv, Wo)
    res = run_bass_kernel_spmd(nc, in_maps, core_ids=list(range(N_CORES)))
    LAST_RESULTS = res
    out = np.zeros((B, S, D), np.float32)
    for core in range(N_CORES):
        out[core // 4] += res.results[core]["y"]
    return out


# revision 32
# speedup vs baseline: 1.0448x; 1.0448x over previous
"""Longformer-style blocked local+global attention on 8 Trainium2 NeuronCores.

Problem (hardcoded): B=2, S=4096, D=1024, H=16, DH=64, W=256 (block size =
one-sided window radius), G=64 global prefix tokens.

Sharding: batch x head-group. Core c handles batch b = c//4 and heads
[(c%4)*4, (c%4)*4+4). Everything for one (b, head-group) is independent, so
there are no collectives; the only cross-core interaction is the final
output-projection partial sum, which the host performs (4 partials per batch).

Per-core kernel layout strategy:
  - Host passes x[b] pre-transposed (xT = [D, S]) so all projection matmuls
    have their contraction dim (D) on SBUF partitions.
  - q, k are produced directly in transposed layout qT/kT = [head_dim, S].
  - v is produced in natural layout [S, DH] per head, with a 65th column of
    ones appended so the PV matmul accumulates the softmax denominator as
    row 64 of the output for free.
  - Scores are computed transposed (sT = [key_pos, query]) so that exp'd
    scores feed the PV matmul directly as the moving operand -- no
    transposes anywhere in the kernel.
  - Softmax uses exp without max subtraction (scores are O(1) by
    construction), and masked entries are zeroed via precomputed 0/1
    triangular mask tiles after exp.
  - All matmul inputs are bf16; accumulation is fp32 in PSUM.

Scheduling (the PE HAM clock gate demotes the PE to 1.2 GHz after ~3.4us of
idle and only re-promotes after ~3.4us of sustained activity, so the tensor
stream must never develop bubbles):
  - Startup: wq is loaded per-k-tile so the first projection matmuls start
    early; wk/wv/wo/masks are single DMAs behind it on the gpsimd queue; each
    xT chunk is split across the sync and scalar queues.
  - Phase 2 per block n emits scores(n,hp0), scores(n,hp1), Wo(n-1),
    PV(n,hp0)+norm, PV(n,hp1)+norm.  Wo and the second PV cover the exp,
    mask and normalize latencies of the first.
  - Score tiles go out c3/c4 first (the first chunks PV consumes); PV runs
    its mask-free/early chunks first and the [c0|c5|glb] tail last.
  - et is [128, ti, hh, 512] so the three per-(hp,hh) triangle masks of a
    generic block collapse into ONE strided 768-col multiply; hh0 runs on
    DVE and hh1 on Pool(gpsimd) to balance the queues.
  - Per-block engine budget vs the PE's 6.8us: ACT 12 exps (~6.9);
    DVE combined-mask hh0 + den copies + reciprocals + normalize muls +
    Wo psum->sbuf bounces (~6.7); Pool combined-mask hh1 + den broadcasts
    (~4); every chain gets >=2.5us of PE cover before it is consumed.
  - Score psum tiles and Wo psum tiles share one 6-buffer pool tag; the two
    PV accumulators double-buffer: 6 + 2 = all 8 psum banks.
"""

import numpy as np
import ml_dtypes

import concourse.bacc as bacc
import concourse.bass as bass
import concourse.mybir as mybir
import concourse.tile as tile
from concourse.bass_utils import run_bass_kernel_spmd

BF16 = mybir.dt.bfloat16
F32 = mybir.dt.float32
NPBF = ml_dtypes.bfloat16

B, S, D = 2, 4096, 1024
H, DH = 16, 64
W = 256          # block size == window radius
G = 64           # global prefix tokens
NB = S // W      # 16 blocks
SCALE = 1.0 / 8.0  # 1/sqrt(DH)

N_CORES = 8
HEADS_PER_CORE = 4
ECOLS = HEADS_PER_CORE * DH   # 256 embedding columns per core

# mask stack indices (each [128, 512] left-aligned, see build_masks)
M_L1, M_R0, M_EGEN, M_EN1, M_GC, M_R0R1, M_L0L1 = range(7)

# module-level caches
_BUILT = {}
LAST_RESULTS = None


def build_masks():
    """Returns (masks [7,128,512], cmasks [2,128,768]) bf16 0/1 tiles.

    Local-strip chunk c of query block n holds key rows kj of block n-1
    (c=0,1), n (c=2,3), n+1 (c=4,5). Triangle masks (r = row within chunk,
    q = query within block): c0: q<=r (only q<128 possible); c1: q<=128+r;
    c4: q>=r; c5: q>=128+r (only q>=128 possible).

    cmasks are the generic-block combined masks, one 256-col slab per score
    tile in emission order [t(c4|c3) | t(c1|c2) | t(c0|c5|glb)]:
    [R0 | L1 | L0h,R1h], with the n==1 variant applying the global cut to
    the c0 triangle.
    """
    r = np.arange(128)[:, None]
    q = np.arange(256)[None, :]
    L0 = (q <= r).astype(np.float32)          # use cols 0:128
    L1 = (q <= 128 + r).astype(np.float32)
    R0 = (q >= r).astype(np.float32)
    R1 = (q >= 128 + r).astype(np.float32)    # use cols 128:256
    L0g = L0 * (r >= G)                       # left-upper chunk w/ global cut
    Gc = np.broadcast_to((r >= G).astype(np.float32), (128, 256)).copy()

    L0h, L0gh, R1h = L0[:, 0:128], L0g[:, 0:128], R1[:, 128:256]

    def pad(*parts):
        m = np.concatenate(parts, axis=1)
        if m.shape[1] < 512:
            m = np.concatenate(
                [m, np.zeros((128, 512 - m.shape[1]), np.float32)], axis=1)
        return m

    m = np.stack([
        pad(L1),             # M_L1
        pad(R0),             # M_R0
        pad(L0h, R1h),       # M_EGEN  [c0|c5]
        pad(L0gh, R1h),      # M_EN1   [c0 w/ global cut|c5] (n==1)
        pad(Gc),             # M_GC    (n==0 c2)
        pad(R0, R1h),        # M_R0R1  (n==0 [c4|c5])
        pad(L0h, L1),        # M_L0L1  (n==15 [c0|c1])
    ]).astype(NPBF)

    egen = np.concatenate([L0h, R1h], axis=1)   # [128, 256]
    en1 = np.concatenate([L0gh, R1h], axis=1)
    cm = np.stack([
        np.concatenate([R0, L1, egen], axis=1),  # generic
        np.concatenate([R0, L1, en1], axis=1),   # n == 1
    ]).astype(NPBF)
    return m, cm


def _tiles_for_block(n):
    """Per-head score-tile packing for query block n.

    Each head's active strip chunks (+ the global-prefix scores) are packed
    into three [128, 512] PSUM banks. Returns (tiles, cmask_idx):
      parts: [(c, col0, width, q0)] -- c in 0..5 or 'glb'; the chunk's
             scores land at psum/et cols [col0, col0+width), covering query
             range [q0, q0+width)
      exps:  [(col0, col1, rows)] -- merged exp ranges
      mask:  (col0, col1, mask_idx) or None -- per-tile mask multiply (edge
             blocks only; generic blocks use the combined cmask instead)
    cmask_idx is None for edge blocks, else the cmasks row to apply over
    et[:, :, hh, 0:256].  Chunk c covers sequence tile s_tile = 2*(n-1)+c.
    Generic blocks emit the c4/c3 tile first (c3 is the first chunk PV
    consumes) with the masked chunk at cols 0:256 so the three masks line
    up for one strided multiply.
    """
    if n == 0:
        return [
            dict(parts=[(2, 0, 256, 0), (3, 256, 256, 0)],
                 exps=[(0, 512, 128)], mask=(0, 256, M_GC)),
            dict(parts=[(4, 0, 256, 0), (5, 256, 128, 128)],
                 exps=[(0, 384, 128)], mask=(0, 384, M_R0R1)),
            dict(parts=[("glb", 0, 256, 0)], exps=[(0, 256, 64)], mask=None),
        ], None
    if n == NB - 1:
        return [
            dict(parts=[(0, 0, 128, 0), (1, 128, 256, 0)],
                 exps=[(0, 384, 128)], mask=(0, 384, M_L0L1)),
            dict(parts=[(2, 0, 256, 0), (3, 256, 256, 0)],
                 exps=[(0, 512, 128)], mask=None),
            dict(parts=[("glb", 0, 256, 0)], exps=[(0, 256, 64)], mask=None),
        ], None
    return [
        dict(parts=[(4, 0, 256, 0), (3, 256, 256, 0)],
             exps=[(0, 512, 128)], mask=None),
        dict(parts=[(1, 0, 256, 0), (2, 256, 256, 0)],
             exps=[(0, 512, 128)], mask=None),
        dict(parts=[(0, 0, 128, 0), (5, 128, 128, 128), ("glb", 256, 256, 0)],
             exps=[(0, 512, 128)], mask=None),
    ], (1 if n == 1 else 0)


def build():
    """Build the per-core Bass/Tile program (identical on all 8 cores)."""
    nc = bacc.Bacc("TRN2", target_bir_lowering=False, debug=False)

    xT = nc.dram_tensor("xT", [D, S], BF16, kind="ExternalInput")
    wq = nc.dram_tensor("wq", [D, ECOLS], BF16, kind="ExternalInput")
    wk = nc.dram_tensor("wk", [D, ECOLS], BF16, kind="ExternalInput")
    wv = nc.dram_tensor("wv", [D, ECOLS], BF16, kind="ExternalInput")
    wo = nc.dram_tensor("wo", [ECOLS, D], BF16, kind="ExternalInput")
    masks = nc.dram_tensor("masks", [7, 128, 512], BF16, kind="ExternalInput")
    cmasks = nc.dram_tensor("cmasks", [2, 128, 768], BF16,
                            kind="ExternalInput")
    y = nc.dram_tensor("y", [S, D], F32, kind="ExternalOutput")

    EXP = mybir.ActivationFunctionType.Exp

    with tile.TileContext(nc) as tc:
        with (
            tc.tile_pool(name="const", bufs=1) as constp,
            tc.tile_pool(name="persist", bufs=1) as pers,
            tc.tile_pool(name="etp", bufs=4) as etp,
            tc.tile_pool(name="attnp", bufs=4) as atp,
            tc.tile_pool(name="smallp", bufs=8) as smp,
            tc.tile_pool(name="yp", bufs=2) as yp,
        ):
            # ---- constants ----
            wq_sb = constp.tile([128, 8, ECOLS], BF16, name="wq_sb")
            wk_sb = constp.tile([128, 8, ECOLS], BF16, name="wk_sb")
            wv_sb = constp.tile([128, 8, ECOLS], BF16, name="wv_sb")
            wo_sb = constp.tile([128, 2, D], BF16, name="wo_sb")
            mk_sb = constp.tile([128, 7, 512], BF16, name="mk_sb")
            cm_sb = constp.tile([128, 2, 3, 256], BF16, name="cm_sb")
            for h in range(2):  # two halves so the first matmuls start early
                nc.gpsimd.dma_start(
                    out=wq_sb[:, h * 4:(h + 1) * 4, :],
                    in_=wq.ap()[h * 512:(h + 1) * 512, :]
                        .rearrange("(k p) e -> p k e", p=128))
            nc.gpsimd.dma_start(
                out=wk_sb[:], in_=wk.ap().rearrange("(k p) e -> p k e", p=128))
            nc.gpsimd.dma_start(
                out=wv_sb[:], in_=wv.ap().rearrange("(k p) e -> p k e", p=128))
            nc.gpsimd.dma_start(
                out=wo_sb[:], in_=wo.ap().rearrange("(e p) d -> p e d", p=128))
            nc.gpsimd.dma_start(
                out=mk_sb[:], in_=masks.ap().rearrange("m p q -> p m q"))
            nc.gpsimd.dma_start(
                out=cm_sb[:],
                in_=cmasks.ap().rearrange("g p (t q) -> p g t q", t=3))

            # ---- persistent per-head tensors ----
            qT = [pers.tile([128, S], BF16, name=f"qT{i}") for i in range(2)]
            kT = [pers.tile([128, S], BF16, name=f"kT{i}") for i in range(2)]
            # v natural layout: [128 seq-part, 32 seq-tiles, 4 heads, 65]
            # (col 64 = ones for the denominator row)
            vv = pers.tile([128, S // 128, HEADS_PER_CORE, 65], BF16, name="vv")
            for h in range(HEADS_PER_CORE):
                nc.vector.memset(vv[:, :, h, 64:65], 1.0)

            # ---- interleaved projections + attention ----
            # Projection chunks and attention blocks share one PE stream so
            # the exp/mask/normalize engines (busy only during attention)
            # spread over the whole kernel instead of cramming into a
            # second phase: block n is emitted as soon as its q/k/v columns
            # (chunks <= (n+1)//2) exist.  All projection psum tiles join
            # the score/Wo rotation (tag "a"): 6 + 2 PV banks = 8.
            with (
                tc.tile_pool(name="xstream", bufs=3) as xp,
                tc.tile_pool(name="ps_s", bufs=6, space="PSUM") as ps_sp,
                tc.tile_pool(name="ps_o", bufs=2, space="PSUM") as ps_op,
            ):
                def emit_chunk_qk(c):
                    xt = xp.tile([128, 8, 512], BF16, name="xt")
                    # split the chunk load across two queues so the first
                    # k-tiles land (and the first matmuls start) early
                    nc.sync.dma_start(
                        out=xt[:, 0:4, :],
                        in_=xT.ap()[0:512, c * 512:(c + 1) * 512]
                            .rearrange("(k p) s -> p k s", p=128))
                    nc.scalar.dma_start(
                        out=xt[:, 4:8, :],
                        in_=xT.ap()[512:1024, c * 512:(c + 1) * 512]
                            .rearrange("(k p) s -> p k s", p=128))
                    for hp in range(2):
                        for wsb, dst in ((wq_sb, qT[hp]), (wk_sb, kT[hp])):
                            ps = ps_sp.tile([128, 512], F32, name="ps_qk",
                                            tag="a")
                            for k in range(8):
                                nc.tensor.matmul(
                                    ps[:],
                                    wsb[:, k, hp * 128:(hp + 1) * 128],
                                    xt[:, k, :],
                                    start=(k == 0), stop=(k == 7))
                            nc.vector.tensor_copy(
                                dst[:, c * 512:(c + 1) * 512], ps[:])
                    return xt

                def emit_chunk_v(c, xt):
                    for ss in range(4):  # 128-row seq subtiles -> v natural
                        ps = ps_sp.tile([128, 512], F32, name="ps_v",
                                        tag="a")
                        for k in range(8):
                            nc.tensor.matmul(
                                ps[:, 0:ECOLS],
                                xt[:, k, ss * 128:(ss + 1) * 128],
                                wv_sb[:, k, :],
                                start=(k == 0), stop=(k == 7))
                        nc.vector.tensor_copy(
                            vv[:, c * 4 + ss, :, 0:64],
                            ps[:, 0:ECOLS].rearrange("p (h e) -> p h e", h=4))

                def emit_chunk(c):
                    emit_chunk_v(c, emit_chunk_qk(c))

                def emit_scores(n, hp, tiles, cidx, et, loc):
                    qpair, kpair = qT[hp], kT[hp]
                    for ti, sp in enumerate(tiles):
                        for hh in range(2):
                            hr = hh * 64
                            st = ps_sp.tile([128, 512], F32, name="st",
                                            tag="a")
                            for c, col0, width, q0 in sp["parts"]:
                                if c == "glb":
                                    lhs = kpair[hr:hr + 64, 0:G]
                                    rows = 64
                                else:
                                    s0 = (2 * (n - 1) + c) * 128
                                    lhs = kpair[hr:hr + 64, s0:s0 + 128]
                                    rows = 128
                                nc.tensor.matmul(
                                    st[0:rows, col0:col0 + width],
                                    lhs,
                                    qpair[hr:hr + 64,
                                          n * 256 + q0:
                                          n * 256 + q0 + width],
                                    start=True, stop=True)
                                if hh == 0:
                                    loc[c] = (ti, col0, width, q0)
                            for c0e, c1e, rows in sp["exps"]:
                                nc.scalar.activation(
                                    et[0:rows, ti, hh, c0e:c1e],
                                    st[0:rows, c0e:c1e], EXP, scale=SCALE)
                        if sp["mask"] is not None:  # edge blocks
                            m0, m1, mi = sp["mask"]
                            for hh in range(2):
                                nc.vector.tensor_mul(
                                    et[:, ti, hh, m0:m1],
                                    et[:, ti, hh, m0:m1],
                                    mk_sb[:, mi, 0:m1 - m0])
                    if cidx is not None:
                        # strided multiplies cover the tiles' triangle
                        # masks; ti0/ti1 split from ti2 so PV's c1/c4
                        # chunks aren't gated on the last exp
                        for hh in range(2):
                            nc.vector.tensor_mul(
                                et[:, 0:2, hh, 0:256],
                                et[:, 0:2, hh, 0:256],
                                cm_sb[:, cidx, 0:2, :])
                        for hh in range(2):
                            nc.vector.tensor_mul(
                                et[:, 2, hh, 0:256],
                                et[:, 2, hh, 0:256],
                                cm_sb[:, cidx, 2, :])

                def emit_pv(n, hp, et, loc, at_blk):
                    # PV + normalize; the two heads share one psum bank.
                    # Mask-free/early chunks first, the t2-dependent tail
                    # (c0/c5/glb) last so the combined mask has PE cover.
                    # psum groups must stay sequential per head: a group
                    # start clears has_written beyond its own address range,
                    # so interleaving the two heads' groups corrupts sums
                    pv_order = [c for c in (3, 2, 1, 4, 0, 5) if c in loc]
                    ot = ps_op.tile([128, 512], F32, name="ot", tag="ot")
                    for hh in range(2):
                        h = hp * 2 + hh
                        ob = hh * 256
                        for i, c in enumerate(pv_order):
                            ti, col0, width, q0 = loc[c]
                            s_tile = 2 * (n - 1) + c
                            nc.tensor.matmul(
                                ot[0:65, ob + q0:ob + q0 + width],
                                vv[:, s_tile, h, :],
                                et[:, ti, hh, col0:col0 + width],
                                start=(i == 0), stop=False)
                        ti, col0, width, q0 = loc["glb"]
                        nc.tensor.matmul(
                            ot[0:65, ob:ob + 256],
                            vv[0:64, 0, h, :],
                            et[0:64, ti, hh, col0:col0 + width],
                            start=False, stop=True)
                    # normalize: reciprocal_approx_fast needs exact fp32
                    # bits; its PSUM read path perturbs them (HW-measured
                    # ~5% error), so bounce the den row through SBUF.
                    den = smp.tile([1, 512], F32, name="den")
                    nc.vector.tensor_copy(den[:], ot[64:65, 0:512])
                    rec = smp.tile([1, 512], F32, name="rec")
                    nc.vector.reciprocal_approx_fast(rec[:], den[:])
                    recb = smp.tile([64, 512], F32, name="recb")
                    nc.gpsimd.partition_broadcast(recb[:], rec[:])
                    for hh in range(2):
                        ob = hh * 256
                        nc.vector.tensor_mul(
                            at_blk[hh * 64:(hh + 1) * 64, hp, :],
                            ot[0:64, ob:ob + 256], recb[:, ob:ob + 256])

                def emit_wo(n, at_blk):
                    """Output projection for block n's 256 rows."""
                    for ss in range(2):
                        ysb = yp.tile([128, D], F32, name="ysb")
                        for dk in range(2):
                            py_ = ps_sp.tile([128, 512], F32, name="py",
                                             tag="a")
                            for e in range(2):
                                nc.tensor.matmul(
                                    py_[:],
                                    at_blk[:, e, ss * 128:(ss + 1) * 128],
                                    wo_sb[:, e, dk * 512:(dk + 1) * 512],
                                    start=(e == 0), stop=(e == 1))
                            nc.vector.tensor_copy(
                                ysb[:, dk * 512:(dk + 1) * 512], py_[:])
                        r0 = n * 256 + ss * 128
                        nc.sync.dma_start(out=y.ap()[r0:r0 + 128, :],
                                          in_=ysb[:])

                # Wo for block n runs between block n+1's two PVs so the
                # in-order PE never stalls on the normalize chain and the
                # second PV's exps/masks get extra cover.
                state = {"pending": None}

                def emit_block(n, mid=None):
                    at_blk = atp.tile([128, 2, 256], BF16, name="at_blk")
                    tiles, cidx = _tiles_for_block(n)
                    loc = {}
                    ets = []
                    for hp in range(2):
                        et = etp.tile([128, 3, 2, 512], BF16, name="et")
                        ets.append(et)
                        emit_scores(n, hp, tiles, cidx, et, loc)
                        if hp == 0 and mid is not None:
                            mid()  # chunk V matmuls drain hp0's exps
                    emit_pv(n, 0, ets[0], loc, at_blk)
                    if state["pending"] is not None:
                        emit_wo(*state["pending"])
                    emit_pv(n, 1, ets[1], loc, at_blk)
                    state["pending"] = (n, at_blk)

                # block n's inputs exist once chunk (n+1)//2 is in.  Blocks
                # sandwich each chunk ([B, c, B]) so the activation engine's
                # exp bursts spread over the chunk's projection matmuls
                # instead of cramming between consecutive blocks.
                emit_chunk(0)
                emit_block(0)
                emit_chunk(1)
                emit_block(1)
                for c in range(2, 8):
                    emit_block(2 * c - 2)   # needs chunk c-1 only
                    xt = emit_chunk_qk(c)
                    # block 2c-1 needs chunk c's q/k for scores but its v
                    # only at PV time: the V matmuls sit between the two
                    # score groups as an exp-drain window
                    emit_block(2 * c - 1,
                               mid=lambda c=c, xt=xt: emit_chunk_v(c, xt))
                emit_block(14)
                emit_block(15)
                emit_wo(*state["pending"])

    nc.compile()
    return nc


def _get_nc():
    if "nc" not in _BUILT:
        _BUILT["nc"] = build()
    return _BUILT["nc"]


def make_in_maps(x, Wq, Wk, Wv, Wo):
    masks_np, cmasks_np = build_masks()
    xT = [np.ascontiguousarray(x[b].T).astype(NPBF) for b in range(B)]
    wq16, wk16, wv16 = (w.astype(NPBF) for w in (Wq, Wk, Wv))
    wo16 = Wo.astype(NPBF)
    in_maps = []
    for core in range(N_CORES):
        b, hg = core // 4, core % 4
        cols = slice(hg * ECOLS, (hg + 1) * ECOLS)
        in_maps.append({
            "xT": xT[b],
            "wq": np.ascontiguousarray(wq16[:, cols]),
            "wk": np.ascontiguousarray(wk16[:, cols]),
            "wv": np.ascontiguousarray(wv16[:, cols]),
            "wo": np.ascontiguousarray(wo16[cols, :]),
            "masks": masks_np,
            "cmasks": cmasks_np,
        })
    return in_maps


def kernel(x, Wq, Wk, Wv, Wo):
    global LAST_RESULTS
    nc = _get_nc()
    in_maps = make_in_maps(x, Wq, Wk, Wv, Wo)
    res = run_bass_kernel_spmd(nc, in_maps, core_ids=list(range(N_CORES)))
    LAST_RESULTS = res
    out = np.zeros((B, S, D), np.float32)
    for core in range(N_CORES):
        out[core // 4] += res.results[core]["y"]
    return out
